# revision 1
# baseline (speedup 1.0000x reference)
"""3-layer GCN (CrystalGCN) on 8 TRN2 NeuronCores.

Strategy (graph/data parallel, nodes sharded by range):
  - 50000 nodes -> 6250/core (padded 6272 = 49 tiles of 128).
  - Edges (incl. self-loops) assigned to the core owning their dst.
  - Per layer l: z = (A_hat @ h) @ W + b  (associativity: aggregate first).
    * gather h[src] rows via gpsimd.dma_gather (bf16, sorted by src,
      lo/hi table split to fit int16 indices),
    * scatter-add via PE matmul: aggT[f, dst] += msgs[e, f].T @ S[e, dst],
      S one-hot * dinv[dst] built on DVE (iota is_equal + mult),
    * dense matmul aggT.T @ W accumulating in PSUM,
    * epilogue relu(z+b) * dinv (pre-scales next layer's gather source).
  - AllGather replicates h between layers; final log_softmax on-chip.

Host preprocessing (numpy) builds index/metadata tensors; the device
kernel is static given the (fixed) edge distribution statistics.
"""
import numpy as np
import ml_dtypes

N = 50000
E = 800000
F_IN, F_HID, F_OUT = 128, 256, 10
F_OUT_P = 16
NCORES = 8
NSH = N // NCORES            # 6250
P = 128
NT = (NSH + P - 1) // P      # 49 node tiles per core
NSHP = NT * P                # 6272 padded shard rows
NROWS = NSHP * NCORES        # 50176 padded global rows
HI_OFF = 17408               # hi table = rows [17408, 50176), 32768 rows
LO_LIM = 32768
BATCH = 4                    # node tiles per gather pair

BF16 = ml_dtypes.bfloat16


def _wrap_idx16(vals):
    """dma_gather index layout: edge i -> [i%16, i//16], replicated to 8
    groups of 16 partitions (one copy per Q7 core)."""
    n = len(vals)
    assert n % 16 == 0
    blk = np.asarray(vals, dtype=np.int16).reshape(n // 16, 16).T
    return np.tile(blk, (8, 1))


def _preprocess(x, edge_index):
    """Build per-core gather/scatter metadata. Returns dict of host arrays."""
    x = np.asarray(x, dtype=np.float32)
    ei = np.asarray(edge_index, dtype=np.int64)
    src_all = np.concatenate([ei[0], np.arange(N, dtype=np.int64)])
    dst_all = np.concatenate([ei[1], np.arange(N, dtype=np.int64)])
    deg = np.bincount(dst_all, minlength=N).astype(np.float32)
    dinv = np.where(deg > 0, 1.0 / np.sqrt(deg), 0.0).astype(np.float32)

    # gather row numbering uses padded global rows
    gidx_all = (src_all // NSH) * NSHP + (src_all % NSH)

    # x gather source: dinv-prescaled, padded layout, bf16
    x_pad = np.zeros((NROWS, F_IN), dtype=BF16)
    xs = (x * dinv[:, None]).astype(BF16)
    for c in range(NCORES):
        x_pad[c * NSHP:c * NSHP + NSH] = xs[c * NSH:(c + 1) * NSH]

    core_of = dst_all // NSH
    tile_of = (dst_all % NSH) // P
    slot_of = (dst_all % NSH) % P

    # per (core, tile) edge lists sorted by gidx
    per = {}
    cnts = np.zeros((NCORES, NT), dtype=np.int64)
    lo_cap = np.zeros((NCORES, NT), dtype=np.int64)
    hi_cap = np.zeros((NCORES, NT), dtype=np.int64)
    order = np.lexsort((gidx_all, tile_of, core_of))
    g_sorted = gidx_all[order]
    slot_sorted = slot_of[order]
    dst_sorted = dst_all[order]
    key = core_of[order] * NT + tile_of[order]
    starts = np.searchsorted(key, np.arange(NCORES * NT))
    ends = np.searchsorted(key, np.arange(NCORES * NT), side="right")
    for c in range(NCORES):
        for t in range(NT):
            k = c * NT + t
            g = g_sorted[starts[k]:ends[k]]
            per[(c, t)] = (g, slot_sorted[starts[k]:ends[k]],
                           dst_sorted[starts[k]:ends[k]])
            cnts[c, t] = len(g)
            lo_cap[c, t] = np.searchsorted(g, LO_LIM)
            hi_cap[c, t] = len(g) - np.searchsorted(g, HI_OFF)

    tl_min = int(np.ceil((cnts - hi_cap).max() / P))
    tl_max = int(lo_cap.min() // P)
    assert tl_min <= tl_max, (tl_min, tl_max)
    # TL=8 makes each lo stream an exact multiple of the 1024-idx
    # dma_gather limit (8 tiles * 128)
    TL = int(np.clip(8, tl_min, tl_max))
    TH = int(np.ceil((cnts.max() - TL * P) / P))
    TNT = TL + TH

    batches = [list(range(i, min(i + BATCH, NT))) for i in range(0, NT, BATCH)]

    cores = []
    for c in range(NCORES):
        dslot = np.zeros((P, NT * TNT), dtype=np.float32)
        dinvd = np.zeros((P, NT * TNT), dtype=np.float32)
        lo_idx_parts = []
        hi_idx_parts = []
        for batch in batches:
            lo_stream = np.zeros(len(batch) * TL * P, dtype=np.int64)
            hi_stream = np.zeros(len(batch) * TH * P, dtype=np.int64)
            for bi, t in enumerate(batch):
                g, sl, dd = per[(c, t)]
                nlo = TL * P
                glo, ghi = g[:nlo], g[nlo:]
                sllo, slhi = sl[:nlo], sl[nlo:]
                ddlo, ddhi = dd[:nlo], dd[nlo:]
                assert glo.max() < LO_LIM and (len(ghi) == 0 or ghi.min() >= HI_OFF)
                lo_stream[bi * nlo:(bi + 1) * nlo] = glo
                nhi = len(ghi)
                hi_stream[bi * TH * P:bi * TH * P + nhi] = ghi - HI_OFF
                # hi dummies stay idx 0 (valid row, dinv 0)
                # metadata: tile j of node-tile t is meta col t*TNT+j
                for m in range(nlo):
                    pass  # vectorized below
                mlo = np.arange(nlo)
                dslot[mlo % P, t * TNT + mlo // P] = sllo
                dinvd[mlo % P, t * TNT + mlo // P] = dinv[ddlo]
                mhi = np.arange(nhi)
                dslot[mhi % P, t * TNT + TL + mhi // P] = slhi
                dinvd[mhi % P, t * TNT + TL + mhi // P] = dinv[ddhi]
            lo_idx_parts.append(_wrap_idx16(lo_stream))
            hi_idx_parts.append(_wrap_idx16(hi_stream - 0))
        idx_lo = np.concatenate(lo_idx_parts, axis=1)
        idx_hi = np.concatenate(hi_idx_parts, axis=1)

        dinv_node = np.zeros((P, NT), dtype=np.float32)
        loc = np.arange(NSH)
        dinv_node[loc % P, loc // P] = dinv[c * NSH:(c + 1) * NSH]

        iota = np.broadcast_to(np.arange(P, dtype=np.float32), (P, P))
        meta = np.concatenate([iota, dslot, dinvd, dinv_node], axis=1).astype(np.float32)
        cores.append({"idx_lo": idx_lo, "idx_hi": idx_hi, "meta": meta})

    return {
        "x_pad": x_pad, "cores": cores, "TL": TL, "TH": TH, "TNT": TNT,
        "batches": batches,
    }


def _build_program(TL, TH, TNT, batches):
    import concourse.bass as bass
    from concourse import bacc
    import concourse.mybir as mybir
    from concourse.tile import TileContext

    dt = mybir.dt
    Alu = mybir.AluOpType
    Act = mybir.ActivationFunctionType
    TE = NT * TNT

    nc = bacc.Bacc(num_devices=NCORES)
    x_pad = nc.dram_tensor("x_pad", [NROWS, F_IN], dt.bfloat16, kind="ExternalInput")
    idx_lo = nc.dram_tensor("idx_lo", [P, NT * TL * 8], dt.int16, kind="ExternalInput")
    idx_hi = nc.dram_tensor("idx_hi", [P, NT * TH * 8], dt.int16, kind="ExternalInput")
    meta = nc.dram_tensor("meta", [P, P + 2 * TE + NT], dt.float32, kind="ExternalInput")
    wts = nc.dram_tensor("wts", [P, 256 + 512 + 32], dt.bfloat16, kind="ExternalInput")
    bias = nc.dram_tensor("bias", [P, 256 + 256 + 16], dt.float32, kind="ExternalInput")
    out_d = nc.dram_tensor("out", [NSHP, F_OUT_P], dt.float32, kind="ExternalOutput")

    with TileContext(nc) as tc:
        with tc.tile_pool(name="const", bufs=1) as cpool, \
             tc.tile_pool(name="msgs", bufs=2) as mpool, \
             tc.tile_pool(name="work", bufs=3) as wpool, \
             tc.tile_pool(name="big", bufs=1) as bigpool, \
             tc.tile_pool(name="ps", bufs=2, space="PSUM") as pspool, \
             tc.tile_pool(name="dram", bufs=1, space="DRAM") as dpool:

            idxlo_sb = cpool.tile([P, NT * TL * 8], dt.int16)
            nc.sync.dma_start(out=idxlo_sb[:], in_=idx_lo[:])
            idxhi_sb = cpool.tile([P, NT * TH * 8], dt.int16)
            nc.sync.dma_start(out=idxhi_sb[:], in_=idx_hi[:])
            meta_sb = cpool.tile([P, P + 2 * TE + NT], dt.float32)
            nc.sync.dma_start(out=meta_sb[:], in_=meta[:])
            wts_sb = cpool.tile([P, 256 + 512 + 32], dt.bfloat16)
            nc.sync.dma_start(out=wts_sb[:], in_=wts[:])
            bias_sb = cpool.tile([P, 256 + 256 + 16], dt.float32)
            nc.sync.dma_start(out=bias_sb[:], in_=bias[:])

            iota_ap = meta_sb[:, 0:P]
            dslot0 = P
            dinvd0 = P + TE
            dinvn0 = P + 2 * TE

            h1_shard = dpool.tile([NSHP, F_HID], dt.bfloat16)
            h2_shard = dpool.tile([NSHP, F_HID], dt.bfloat16)
            h1_full = dpool.tile([NROWS, F_HID], dt.bfloat16, addr_space="Shared")
            h2_full = dpool.tile([NROWS, F_HID], dt.bfloat16, addr_space="Shared")

            w_chunks = {
                1: [wts_sb[:, 0:256]],
                2: [wts_sb[:, 256:512], wts_sb[:, 512:768]],
                3: [wts_sb[:, 768:784], wts_sb[:, 784:800]],
            }
            b_tiles = {1: bias_sb[:, 0:256], 2: bias_sb[:, 256:512],
                       3: bias_sb[:, 512:528]}

            def layer(l, gsrc, F_in, nF, F_out, h_big, last):
                for b, batch in enumerate(batches):
                    nb = len(batch)
                    msl = mpool.tile([P, nb * TL, F_in], dt.bfloat16,
                                     tag="msl", bufs=2, name=f"msl_{l}_{b}")
                    msh = mpool.tile([P, nb * TH, F_in], dt.bfloat16,
                                     tag="msh", bufs=2, name=f"msh_{l}_{b}")
                    c0 = batch[0]
                    # dma_gather caps at 1024 idxs/call -> chunk by 8 tiles
                    for off in range(0, nb * TL, 8):
                        ct = min(8, nb * TL - off)
                        nc.gpsimd.dma_gather(
                            out_ap=msl[:, off:off + ct, :],
                            in_ap=gsrc[0:LO_LIM, :],
                            idxs_ap=idxlo_sb[:, c0 * TL * 8 + off * 8:
                                             c0 * TL * 8 + (off + ct) * 8],
                            num_idxs=ct * P, num_idxs_reg=ct * P,
                            elem_size=F_in)
                    for off in range(0, nb * TH, 8):
                        ct = min(8, nb * TH - off)
                        nc.gpsimd.dma_gather(
                            out_ap=msh[:, off:off + ct, :],
                            in_ap=gsrc[HI_OFF:HI_OFF + LO_LIM, :],
                            idxs_ap=idxhi_sb[:, c0 * TH * 8 + off * 8:
                                             c0 * TH * 8 + (off + ct) * 8],
                            num_idxs=ct * P, num_idxs_reg=ct * P,
                            elem_size=F_in)
                    for bi, nt in enumerate(batch):
                        aggps = [pspool.tile([P, P], dt.float32, space="PSUM",
                                             tag=f"agg{fc}", bufs=2,
                                             name=f"agg_{l}_{nt}_{fc}")
                                 for fc in range(nF)]
                        for j in range(TNT):
                            g = nt * TNT + j
                            s_t = wpool.tile([P, P], dt.bfloat16, tag="s_t",
                                             bufs=4, name=f"s_{l}_{nt}_{j}")
                            nc.vector.tensor_scalar(
                                out=s_t[:], in0=iota_ap,
                                scalar1=meta_sb[:, dslot0 + g:dslot0 + g + 1],
                                scalar2=meta_sb[:, dinvd0 + g:dinvd0 + g + 1],
                                op0=Alu.is_equal, op1=Alu.mult)
                            if j < TL:
                                m_ap = msl[:, bi * TL + j, :]
                            else:
                                m_ap = msh[:, bi * TH + (j - TL), :]
                            for fc in range(nF):
                                nc.tensor.matmul(
                                    aggps[fc][:],
                                    lhsT=m_ap[:, fc * P:(fc + 1) * P],
                                    rhs=s_t[:],
                                    start=(j == 0), stop=(j == TNT - 1))
                        zps = pspool.tile([P, F_out], dt.float32, space="PSUM",
                                          tag="z", bufs=2, name=f"z_{l}_{nt}")
                        for fc in range(nF):
                            aggsb = wpool.tile([P, P], dt.bfloat16, tag="aggsb",
                                               bufs=3, name=f"aggsb_{l}_{nt}_{fc}")
                            nc.scalar.copy(out=aggsb[:], in_=aggps[fc][:])
                            nc.tensor.matmul(zps[:], lhsT=aggsb[:],
                                             rhs=w_chunks[l][fc],
                                             start=(fc == 0), stop=(fc == nF - 1))
                        tmp = wpool.tile([P, F_out], dt.float32, tag="tmp",
                                         bufs=3, name=f"tmp_{l}_{nt}")
                        nc.vector.tensor_tensor(out=tmp[:], in0=zps[:],
                                                in1=b_tiles[l], op=Alu.add)
                        if not last:
                            nc.scalar.activation(
                                out=h_big[:, nt, :], in_=tmp[:], func=Act.Relu,
                                scale=meta_sb[:, dinvn0 + nt:dinvn0 + nt + 1])
                        else:
                            mx = wpool.tile([P, 1], dt.float32, tag="mx",
                                            bufs=3, name=f"mx_{nt}")
                            nc.vector.tensor_reduce(
                                out=mx[:], in_=tmp[:, 0:F_OUT],
                                axis=mybir.AxisListType.X, op=Alu.max,
                                negate=True)
                            ex = wpool.tile([P, F_OUT], dt.float32, tag="ex",
                                            bufs=3, name=f"ex_{nt}")
                            nc.scalar.activation(out=ex[:], in_=tmp[:, 0:F_OUT],
                                                 func=Act.Exp, bias=mx[:])
                            sm = wpool.tile([P, 1], dt.float32, tag="sm",
                                            bufs=3, name=f"sm_{nt}")
                            nc.vector.tensor_reduce(
                                out=sm[:], in_=ex[:],
                                axis=mybir.AxisListType.X, op=Alu.add)
                            ls = wpool.tile([P, 1], dt.float32, tag="ls",
                                            bufs=3, name=f"ls_{nt}")
                            nc.scalar.activation(out=ls[:], in_=sm[:],
                                                 func=Act.Ln)
                            nls = wpool.tile([P, 1], dt.float32, tag="nls",
                                             bufs=3, name=f"nls_{nt}")
                            nc.vector.tensor_scalar(
                                out=nls[:], in0=ls[:], scalar1=-1.0,
                                scalar2=None, op0=Alu.mult)
                            nc.vector.tensor_scalar(
                                out=h_big[:, nt, 0:F_OUT], in0=tmp[:, 0:F_OUT],
                                scalar1=mx[:], scalar2=nls[:],
                                op0=Alu.add, op1=Alu.add)

            # Layer 1: gather x (128-wide)
            h1_big = bigpool.tile([P, NT, F_HID], dt.bfloat16)
            layer(1, x_pad, F_IN, 1, F_HID, h1_big, last=False)
            nc.sync.dma_start(
                out=h1_shard[:].rearrange("(t p) f -> p t f", p=P),
                in_=h1_big[:])
            nc.gpsimd.collective_compute(
                "AllGather", mybir.AluOpType.bypass,
                replica_groups=[list(range(NCORES))],
                ins=[h1_shard[:].opt()], outs=[h1_full[:].opt()])

            h2_big = bigpool.tile([P, NT, F_HID], dt.bfloat16)
            layer(2, h1_full, F_HID, 2, F_HID, h2_big, last=False)
            nc.sync.dma_start(
                out=h2_shard[:].rearrange("(t p) f -> p t f", p=P),
                in_=h2_big[:])
            nc.gpsimd.collective_compute(
                "AllGather", mybir.AluOpType.bypass,
                replica_groups=[list(range(NCORES))],
                ins=[h2_shard[:].opt()], outs=[h2_full[:].opt()])

            out_big = bigpool.tile([P, NT, F_OUT_P], dt.float32)
            nc.vector.memset(out_big[:], 0.0)
            layer(3, h2_full, F_HID, 2, F_OUT_P, out_big, last=True)
            nc.sync.dma_start(
                out=out_d[:].rearrange("(t p) f -> p t f", p=P),
                in_=out_big[:])

    nc.finalize()
    return nc


_CACHE = {}


def kernel(x, edge_index, W1, b1, W2, b2, W3, b3):
    from concourse.bass_utils import run_bass_kernel_spmd

    prep = _preprocess(x, edge_index)
    TL, TH, TNT = prep["TL"], prep["TH"], prep["TNT"]

    key = (TL, TH)
    if key not in _CACHE:
        _CACHE[key] = _build_program(TL, TH, TNT, prep["batches"])
    nc = _CACHE[key]

    W1 = np.asarray(W1, np.float32)
    W2 = np.asarray(W2, np.float32)
    W3 = np.asarray(W3, np.float32)
    wts = np.zeros((P, 256 + 512 + 32), dtype=BF16)
    wts[:, 0:256] = W1.astype(BF16)
    wts[:, 256:512] = W2[0:128].astype(BF16)
    wts[:, 512:768] = W2[128:256].astype(BF16)
    wts[:, 768:778] = W3[0:128].astype(BF16)
    wts[:, 784:794] = W3[128:256].astype(BF16)
    bias = np.zeros((P, 256 + 256 + 16), dtype=np.float32)
    bias[:, 0:256] = np.asarray(b1, np.float32)[None, :]
    bias[:, 256:512] = np.asarray(b2, np.float32)[None, :]
    bias[:, 512:522] = np.asarray(b3, np.float32)[None, :]

    in_maps = []
    for c in range(NCORES):
        m = dict(prep["cores"][c])
        m["x_pad"] = prep["x_pad"]
        m["wts"] = wts
        m["bias"] = bias
        in_maps.append(m)

    res = run_bass_kernel_spmd(nc, in_maps, core_ids=list(range(NCORES)))
    out = np.zeros((N, F_OUT), dtype=np.float32)
    for c in range(NCORES):
        out[c * NSH:(c + 1) * NSH] = res.results[c]["out"][:NSH, :F_OUT]
    return out

